# revision 10
# baseline (speedup 1.0000x reference)
"""Trainium2 Bass kernel for the noisy-RNN recurrence problem.

Reference computation (per step t):
    tmp   = x_t @ w_in^T + tanh(h) @ w_hh^T + b
    h_new = (1-a)*h + a*tmp + noise_t * (sigma*sqrt(a))
    out   = 20*tanh(h_new @ w_out^T)
Outputs: (hidden_list [B,T,NHID], output_list [B,T,NOUT], hidden_final [B,NHID])

Strategy:
  * Data-parallel over batch: 8 cores x 32 batch each.
  * Host pre-folds u_t = a*(x_t @ w_in^T) + a*b + sigma*sqrt(a)*noise_t into one
    fp32 stream (the dominant memory traffic), pre-transposed into the on-chip
    layout so no on-device transposes are needed.
  * On device, state h^T is kept as a [128 part, (chain, m, b)] fp32 tile
    group; per step: tanh (ScalarE, fp16 out) -> 4 matmuls (fp16 weights
    stationary, fp32 PSUM) -> psum+u (DVE) -> h_new = 0.75*h + t1 (DVE fused
    scalar_tensor_tensor).  Two independent 16-batch chains per core pipeline
    the serial recurrence across engines.
  * output_list and hidden_final are pure functions of hidden_list; computed
    on host (fp32) to keep device traffic at the memory roofline.
"""

import os
import sys

import numpy as np

for _p in ("/opt/trn_rl_repo",):
    if _p not in sys.path:
        sys.path.insert(0, _p)

import concourse.bass as bass  # noqa: E402
import concourse.tile as tile  # noqa: E402
from concourse import bacc, mybir  # noqa: E402
from concourse.bass_utils import run_bass_kernel_spmd  # noqa: E402

ALPHA = np.float32(0.25)
SIGMA_NEU = np.float32(0.05)
N_CORES = 8
W = 16  # timesteps per DMA chunk

_BUILD_CACHE = {}


def _install_profile_hook():
    """Provide antenv.axon_hooks (NTFF profiling) if the image lacks it."""
    try:
        import antenv.axon_hooks  # noqa: F401

        return
    except ImportError:
        pass
    import contextlib
    import ctypes
    import types

    so = "/opt/axon/libaxon_pjrt.so"
    if not os.path.exists(so):
        return
    lib = ctypes.CDLL(so)
    if not hasattr(lib, "axon_start_nrt_profile"):
        return
    lib.axon_start_nrt_profile.argtypes = [
        ctypes.POINTER(ctypes.c_int64),
        ctypes.c_size_t,
    ]
    lib.axon_start_nrt_profile.restype = ctypes.c_int64
    lib.axon_stop_nrt_profile.argtypes = [ctypes.c_char_p]
    lib.axon_stop_nrt_profile.restype = ctypes.c_int64

    @contextlib.contextmanager
    def _hook(output_dir, device_ids):
        import jax

        jax.devices()
        if device_ids:
            ids = (ctypes.c_int64 * len(device_ids))(*device_ids)
            rc = lib.axon_start_nrt_profile(ids, len(device_ids))
        else:
            rc = lib.axon_start_nrt_profile(None, 0)
        if rc != 0:
            raise RuntimeError(f"axon_start_nrt_profile rc={rc}")
        try:
            yield
        finally:
            n = lib.axon_stop_nrt_profile(str(output_dir).encode())
            print(f"profile: {n} file(s) written to {output_dir}", file=sys.stderr)

    import antenv

    mod = types.ModuleType("antenv.axon_hooks")
    mod.get_axon_ntff_profile_hook = lambda: _hook
    mod.set_axon_ntff_profile_hook = lambda h: None
    antenv.axon_hooks = mod
    sys.modules["antenv.axon_hooks"] = mod


S_FP16 = True  # dtype of the s = 0.75*h + u path fed back via identity-matmul


def _build(T, W):
    """Build + compile the per-core bass program (SPMD across 8 cores).

    Design: the state h lives in PSUM. One chain over the full 32-batch,
    tile free layout (m, b) = m*32+b (nhid half m, batch b).
    Per step t (psum slice `sl` holds h_t):
      - ScalarE: th_t = tanh(psum[sl]) -> fp16        (parallel with DVE)
      - DVE:     s_t  = 0.75*psum[sl] + u_t -> fp16/f32
      - TensorE: psum[sl+1] = sum_k aW^T_k @ th_t  +  I @ s_t
    Every 8 steps one PSUM bank completes -> DVE copies it to SBUF hist,
    DMA'd out every 16 steps. hidden_list <- hist; output head on host.
    """
    nchunk = T // W
    assert W == 16
    f32, f16 = mybir.dt.float32, mybir.dt.float16
    sdt = f16 if S_FP16 else f32
    nc = bacc.Bacc("TRN2", target_bir_lowering=False, debug=False)
    u_d = nc.dram_tensor("u", [nchunk, 128, W * 64], f32, kind="ExternalInput").ap()
    h0_d = nc.dram_tensor("h0", [128, 64], f32, kind="ExternalInput").ap()
    wt_d = nc.dram_tensor("wt", [128, 512], f16, kind="ExternalInput").ap()
    id_d = nc.dram_tensor("ident", [128, 128], sdt, kind="ExternalInput").ap()
    ho_d = nc.dram_tensor("hout", [nchunk, 128, W * 64], f32, kind="ExternalOutput").ap()

    Tanh = mybir.ActivationFunctionType.Tanh
    add, mult = mybir.AluOpType.add, mybir.AluOpType.mult

    with tile.TileContext(nc) as tc:
        with (
            tc.tile_pool(name="const", bufs=1) as constp,
            tc.tile_pool(name="upool", bufs=3) as upool,
            tc.tile_pool(name="hist", bufs=3) as histp,
            tc.tile_pool(name="th", bufs=3) as thp,
            tc.tile_pool(name="sp", bufs=3) as sp,
            tc.tile_pool(name="ps", bufs=6, space="PSUM") as psp,
        ):
            wt_s = constp.tile([128, 512], f16)
            nc.sync.dma_start(wt_s[:], wt_d[:, :])
            id_s = constp.tile([128, 128], sdt)
            nc.sync.dma_start(id_s[:], id_d[:, :])
            h0_s = constp.tile([128, 64], f32)
            nc.sync.dma_start(h0_s[:], h0_d[:, :])

            # One PSUM bank tile [128,64] per step: step t writes h_{t+1}
            # into its own bank (w-matmuls first - gated only on th_t - then
            # the identity-matmul injecting s_t last).  Readers of bank t
            # (tanh, stt, hist-copy) never share a bank with the writers of
            # bank t+1, so PE writes overlap the reads of the previous step.
            read_ap = h0_s[:]
            u_s = None
            hist = None
            # (src_ap, hist_tile, col, dma_chunk|None) for the hist-copy of
            # the previous step - emitted AFTER the next step's tanh/stt so
            # the copy is last in the PSUM bank's reader order (Tile
            # serializes same-bank accessors in program order).
            pend = None

            def flush_pend():
                src, ht, col, dma_chunk = pend
                nc.vector.tensor_copy(ht[:, col : col + 64], src)
                if dma_chunk is not None:
                    nc.sync.dma_start(ho_d[dma_chunk], ht[:])

            for t in range(T):
                w16 = t % 16
                if w16 == 0:
                    u_s = upool.tile([128, W * 64], f32)
                    nc.sync.dma_start(u_s[:], u_d[t // W])
                    hist = histp.tile([128, W * 64], f32)
                th = thp.tile([128, 64], f16)
                nc.scalar.activation(th[:], read_ap, Tanh)
                s = sp.tile([128, 64], sdt)
                nc.vector.scalar_tensor_tensor(
                    s[:], read_ap, 0.75, u_s[:, w16 * 64 : (w16 + 1) * 64], mult, add
                )
                if pend is not None:
                    flush_pend()
                ps = psp.tile([128, 64], f32)
                for m in range(2):
                    for k in range(2):
                        nc.tensor.matmul(
                            ps[:, m * 32 : (m + 1) * 32],
                            wt_s[:, k * 256 + m * 128 : k * 256 + (m + 1) * 128],
                            th[:, k * 32 : (k + 1) * 32],
                            start=(m == 0 and k == 0),
                            stop=False,
                            skip_group_check=True,
                        )
                nc.tensor.matmul(
                    ps[:], id_s[:], s[:],
                    start=False, stop=True, skip_group_check=True,
                )
                read_ap = ps[:]
                pend = (ps[:], hist, w16 * 64, t // W if w16 == 15 else None)
            flush_pend()
    nc.compile()
    return nc


def _get_nc(T, W):
    key = (T, W)
    if key not in _BUILD_CACHE:
        _BUILD_CACHE[key] = _build(T, W)
    return _BUILD_CACHE[key]


def kernel(input_signal, hidden, w_in_w, w_hh_w, w_hh_b, w_out_w, noise, length):
    input_signal = np.asarray(input_signal, dtype=np.float32)
    hidden = np.asarray(hidden, dtype=np.float32)
    w_in_w = np.asarray(w_in_w, dtype=np.float32)
    w_hh_w = np.asarray(w_hh_w, dtype=np.float32)
    w_hh_b = np.asarray(w_hh_b, dtype=np.float32)
    w_out_w = np.asarray(w_out_w, dtype=np.float32)
    noise = np.asarray(noise, dtype=np.float32)

    B, T, NIN = input_signal.shape
    NHID = hidden.shape[1]
    NOUT = w_out_w.shape[0]
    BC = B // N_CORES  # 32 batch per core
    nchunk = T // W
    sigp = SIGMA_NEU * np.sqrt(ALPHA)

    # ---- host: fold input projection + bias + noise into one fp32 stream ----
    # u_full[t, b, :] = a*(x[b,t,:] @ w_in^T) + a*b_hh + sigp*noise[t,b,:]
    xt = np.ascontiguousarray(input_signal.transpose(1, 0, 2)).reshape(T * B, NIN)
    proj = (xt @ (ALPHA * w_in_w.T)).reshape(T, B, NHID)
    u_full = proj + (ALPHA * w_hh_b)[None, None, :] + sigp * noise
    u_full = np.ascontiguousarray(u_full, dtype=np.float32)

    # weights, stationary layout: wt_s[p, k*256 + j] = (a*w_hh^T)[k*128+p, j]
    wt = (ALPHA * w_hh_w.T).astype(np.float16)  # [256 (i=contraction), 256 (j=out)]
    wt_s = np.ascontiguousarray(
        wt.reshape(2, 128, 256).transpose(1, 0, 2).reshape(128, 512)
    )

    ident = np.eye(128, dtype=np.float16 if S_FP16 else np.float32)
    in_maps = []
    for c in range(N_CORES):
        # u: (chunk, p, (w, m, b))
        uc = u_full[:, c * BC : (c + 1) * BC, :]  # [T, 32, 256]
        uc = uc.reshape(nchunk, W, BC, 2, 128)  # (chunk, w, b, m, p)
        uc = np.ascontiguousarray(uc.transpose(0, 4, 1, 3, 2)).reshape(
            nchunk, 128, W * 64
        )
        # h0: (p, (m, b))
        hc = hidden[c * BC : (c + 1) * BC, :]  # [32, 256] dims (b, (m, p))
        hc = hc.reshape(BC, 2, 128).transpose(2, 1, 0)  # (p, m, b)
        hc = np.ascontiguousarray(hc).reshape(128, 64)
        in_maps.append({"u": uc, "h0": hc, "wt": wt_s, "ident": ident})

    trace = bool(int(os.environ.get("KERNEL_TRACE", "0")))
    if trace:
        _install_profile_hook()
    nc = _get_nc(T, W)
    res = run_bass_kernel_spmd(
        nc,
        in_maps,
        core_ids=list(range(N_CORES)),
        trace=trace,
    )
    kernel.last_results = res

    # ---- host: unpack hidden_list, compute output head ----
    hidden_list = np.empty((B, T, NHID), dtype=np.float32)
    for c in range(N_CORES):
        arr = res.results[c]["hout"].reshape(nchunk, 128, W, 2, BC)
        # (chunk, p, w, m, b) -> (b, chunk, w, m, p)
        arr = arr.transpose(4, 0, 2, 3, 1).reshape(BC, T, NHID)
        hidden_list[c * BC : (c + 1) * BC] = arr

    hidden_final = np.ascontiguousarray(hidden_list[:, -1, :])
    out = hidden_list.reshape(B * T, NHID) @ w_out_w.T.astype(np.float32)
    output_list = (20.0 * np.tanh(out)).reshape(B, T, NOUT).astype(np.float32)
    return hidden_list, output_list, hidden_final


# revision 12
# speedup vs baseline: 1.0138x; 1.0138x over previous
"""Trainium2 Bass kernel for the noisy-RNN recurrence problem.

Reference computation (per step t):
    tmp   = x_t @ w_in^T + tanh(h) @ w_hh^T + b
    h_new = (1-a)*h + a*tmp + noise_t * (sigma*sqrt(a))
    out   = 20*tanh(h_new @ w_out^T)
Outputs: (hidden_list [B,T,NHID], output_list [B,T,NOUT], hidden_final [B,NHID])

Strategy:
  * Data-parallel over batch: 8 cores x 32 batch each.
  * Host pre-folds u_t = a*(x_t @ w_in^T) + a*b + sigma*sqrt(a)*noise_t into one
    fp32 stream (the dominant memory traffic), pre-transposed into the on-chip
    layout so no on-device transposes are needed.
  * On device, state h^T is kept as a [128 part, (chain, m, b)] fp32 tile
    group; per step: tanh (ScalarE, fp16 out) -> 4 matmuls (fp16 weights
    stationary, fp32 PSUM) -> psum+u (DVE) -> h_new = 0.75*h + t1 (DVE fused
    scalar_tensor_tensor).  Two independent 16-batch chains per core pipeline
    the serial recurrence across engines.
  * output_list and hidden_final are pure functions of hidden_list; computed
    on host (fp32) to keep device traffic at the memory roofline.
"""

import os
import sys

import numpy as np

for _p in ("/opt/trn_rl_repo",):
    if _p not in sys.path:
        sys.path.insert(0, _p)

import concourse.bass as bass  # noqa: E402
import concourse.tile as tile  # noqa: E402
from concourse import bacc, mybir  # noqa: E402
from concourse.bass_utils import run_bass_kernel_spmd  # noqa: E402

ALPHA = np.float32(0.25)
SIGMA_NEU = np.float32(0.05)
N_CORES = 8
W = 16  # timesteps per DMA chunk

_BUILD_CACHE = {}


def _install_profile_hook():
    """Provide antenv.axon_hooks (NTFF profiling) if the image lacks it."""
    try:
        import antenv.axon_hooks  # noqa: F401

        return
    except ImportError:
        pass
    import contextlib
    import ctypes
    import types

    so = "/opt/axon/libaxon_pjrt.so"
    if not os.path.exists(so):
        return
    lib = ctypes.CDLL(so)
    if not hasattr(lib, "axon_start_nrt_profile"):
        return
    lib.axon_start_nrt_profile.argtypes = [
        ctypes.POINTER(ctypes.c_int64),
        ctypes.c_size_t,
    ]
    lib.axon_start_nrt_profile.restype = ctypes.c_int64
    lib.axon_stop_nrt_profile.argtypes = [ctypes.c_char_p]
    lib.axon_stop_nrt_profile.restype = ctypes.c_int64

    @contextlib.contextmanager
    def _hook(output_dir, device_ids):
        import jax

        jax.devices()
        if device_ids:
            ids = (ctypes.c_int64 * len(device_ids))(*device_ids)
            rc = lib.axon_start_nrt_profile(ids, len(device_ids))
        else:
            rc = lib.axon_start_nrt_profile(None, 0)
        if rc != 0:
            raise RuntimeError(f"axon_start_nrt_profile rc={rc}")
        try:
            yield
        finally:
            n = lib.axon_stop_nrt_profile(str(output_dir).encode())
            print(f"profile: {n} file(s) written to {output_dir}", file=sys.stderr)

    import antenv

    mod = types.ModuleType("antenv.axon_hooks")
    mod.get_axon_ntff_profile_hook = lambda: _hook
    mod.set_axon_ntff_profile_hook = lambda h: None
    antenv.axon_hooks = mod
    sys.modules["antenv.axon_hooks"] = mod


S_FP16 = True  # dtype of the s = 0.75*h + u path fed back via identity-matmul


def _build(T, W):
    """Build + compile the per-core bass program (SPMD across 8 cores).

    Design: the state h lives in PSUM. One chain over the full 32-batch,
    tile free layout (m, b) = m*32+b (nhid half m, batch b).
    Per step t (psum slice `sl` holds h_t):
      - ScalarE: th_t = tanh(psum[sl]) -> fp16        (parallel with DVE)
      - DVE:     s_t  = 0.75*psum[sl] + u_t -> fp16/f32
      - TensorE: psum[sl+1] = sum_k aW^T_k @ th_t  +  I @ s_t
    Every 8 steps one PSUM bank completes -> DVE copies it to SBUF hist,
    DMA'd out every 16 steps. hidden_list <- hist; output head on host.
    """
    nchunk = T // W
    assert W == 16
    f32, f16 = mybir.dt.float32, mybir.dt.float16
    sdt = f16 if S_FP16 else f32
    nc = bacc.Bacc("TRN2", target_bir_lowering=False, debug=False)
    u_d = nc.dram_tensor("u", [nchunk, 128, W * 64], f32, kind="ExternalInput").ap()
    h0_d = nc.dram_tensor("h0", [128, 64], f32, kind="ExternalInput").ap()
    wt_d = nc.dram_tensor("wt", [128, 512], f16, kind="ExternalInput").ap()
    id_d = nc.dram_tensor("ident", [128, 128], sdt, kind="ExternalInput").ap()
    ho_d = nc.dram_tensor("hout", [nchunk, 128, W * 64], f32, kind="ExternalOutput").ap()

    Tanh = mybir.ActivationFunctionType.Tanh
    add, mult = mybir.AluOpType.add, mybir.AluOpType.mult

    with tile.TileContext(nc) as tc:
        with (
            tc.tile_pool(name="const", bufs=1) as constp,
            tc.tile_pool(name="upool", bufs=3) as upool,
            tc.tile_pool(name="hist", bufs=3) as histp,
            tc.tile_pool(name="th", bufs=3) as thp,
            tc.tile_pool(name="sp", bufs=3) as sp,
            tc.tile_pool(name="zp", bufs=3) as zp,
            tc.tile_pool(name="ps", bufs=3, space="PSUM") as psp,
            tc.tile_pool(name="pa", bufs=3, space="PSUM") as pap,
        ):
            wt_s = constp.tile([128, 512], f16)
            nc.sync.dma_start(wt_s[:], wt_d[:, :])
            id_s = constp.tile([128, 128], sdt)
            nc.sync.dma_start(id_s[:], id_d[:, :])
            h0_s = constp.tile([128, 64], f32)
            nc.sync.dma_start(h0_s[:], h0_d[:, :])

            # One PSUM bank tile [128,64] per step: step t writes h_{t+1}
            # into its own bank (w-matmuls first - gated only on th_t - then
            # the identity-matmul injecting s_t last).  Readers of bank t
            # (tanh, stt, hist-copy) never share a bank with the writers of
            # bank t+1, so PE writes overlap the reads of the previous step.
            # Two PSUM banks per step: bank_H[i] = h_{i+1} (I-mm of s_i first,
            # then 4 w-matmuls of th_i); bank_A[i] = A_{i+1} (w-matmuls only,
            # lower priority).  ScalarE's tanh reads bank_H, VectorE's s-path
            # reads bank_A - different banks, so they run in parallel instead
            # of being serialized by Tile's same-bank ordering.
            #   z_i = 0.75*s_{i-1} + u_i          (DVE, SBUF only, off-chain)
            #   s_i = 0.75*A_i + z_i              (DVE, reads bank_A[i-1])
            read_ap = h0_s[:]
            a_prev = None
            s_prev = None
            u_s = None
            hist = None
            pend = None

            def flush_pend():
                src, ht, col, dma_chunk = pend
                nc.vector.tensor_copy(ht[:, col : col + 64], src)
                if dma_chunk is not None:
                    nc.sync.dma_start(ho_d[dma_chunk], ht[:])

            for t in range(T):
                w16 = t % 16
                if w16 == 0:
                    u_s = upool.tile([128, W * 64], f32)
                    nc.sync.dma_start(u_s[:], u_d[t // W])
                    hist = histp.tile([128, W * 64], f32)
                th = thp.tile([128, 64], f16)
                nc.scalar.activation(th[:], read_ap, Tanh)
                u_ap = u_s[:, w16 * 64 : (w16 + 1) * 64]
                s = sp.tile([128, 64], sdt)
                if t == 0:
                    nc.vector.scalar_tensor_tensor(
                        s[:], h0_s[:], 0.75, u_ap, mult, add
                    )
                else:
                    z = zp.tile([128, 64], f32)
                    nc.vector.scalar_tensor_tensor(
                        z[:], s_prev[:], 0.75, u_ap, mult, add
                    )
                    nc.vector.scalar_tensor_tensor(
                        s[:], a_prev[:], 0.75, z[:], mult, add
                    )
                if pend is not None:
                    flush_pend()
                ps = psp.tile([128, 64], f32)
                nc.tensor.matmul(
                    ps[:], id_s[:], s[:],
                    start=True, stop=False, skip_group_check=True,
                )
                for m in range(2):
                    for k in range(2):
                        nc.tensor.matmul(
                            ps[:, m * 32 : (m + 1) * 32],
                            wt_s[:, k * 256 + m * 128 : k * 256 + (m + 1) * 128],
                            th[:, k * 32 : (k + 1) * 32],
                            start=False,
                            stop=(m == 1 and k == 1),
                            skip_group_check=True,
                        )
                if t < T - 1:
                    pa = pap.tile([128, 64], f32)
                    for m in range(2):
                        for k in range(2):
                            nc.tensor.matmul(
                                pa[:, m * 32 : (m + 1) * 32],
                                wt_s[:, k * 256 + m * 128 : k * 256 + (m + 1) * 128],
                                th[:, k * 32 : (k + 1) * 32],
                                start=(k == 0),
                                stop=(k == 1),
                                skip_group_check=True,
                            )
                    a_prev = pa
                s_prev = s
                read_ap = ps[:]
                pend = (ps[:], hist, w16 * 64, t // W if w16 == 15 else None)
            flush_pend()
    nc.compile()
    return nc


def _get_nc(T, W):
    key = (T, W)
    if key not in _BUILD_CACHE:
        _BUILD_CACHE[key] = _build(T, W)
    return _BUILD_CACHE[key]


def kernel(input_signal, hidden, w_in_w, w_hh_w, w_hh_b, w_out_w, noise, length):
    input_signal = np.asarray(input_signal, dtype=np.float32)
    hidden = np.asarray(hidden, dtype=np.float32)
    w_in_w = np.asarray(w_in_w, dtype=np.float32)
    w_hh_w = np.asarray(w_hh_w, dtype=np.float32)
    w_hh_b = np.asarray(w_hh_b, dtype=np.float32)
    w_out_w = np.asarray(w_out_w, dtype=np.float32)
    noise = np.asarray(noise, dtype=np.float32)

    B, T, NIN = input_signal.shape
    NHID = hidden.shape[1]
    NOUT = w_out_w.shape[0]
    BC = B // N_CORES  # 32 batch per core
    nchunk = T // W
    sigp = SIGMA_NEU * np.sqrt(ALPHA)

    # ---- host: fold input projection + bias + noise into one fp32 stream ----
    # u_full[t, b, :] = a*(x[b,t,:] @ w_in^T) + a*b_hh + sigp*noise[t,b,:]
    xt = np.ascontiguousarray(input_signal.transpose(1, 0, 2)).reshape(T * B, NIN)
    proj = (xt @ (ALPHA * w_in_w.T)).reshape(T, B, NHID)
    u_full = proj + (ALPHA * w_hh_b)[None, None, :] + sigp * noise
    u_full = np.ascontiguousarray(u_full, dtype=np.float32)

    # weights, stationary layout: wt_s[p, k*256 + j] = (a*w_hh^T)[k*128+p, j]
    wt = (ALPHA * w_hh_w.T).astype(np.float16)  # [256 (i=contraction), 256 (j=out)]
    wt_s = np.ascontiguousarray(
        wt.reshape(2, 128, 256).transpose(1, 0, 2).reshape(128, 512)
    )

    ident = np.eye(128, dtype=np.float16 if S_FP16 else np.float32)
    in_maps = []
    for c in range(N_CORES):
        # u: (chunk, p, (w, m, b))
        uc = u_full[:, c * BC : (c + 1) * BC, :]  # [T, 32, 256]
        uc = uc.reshape(nchunk, W, BC, 2, 128)  # (chunk, w, b, m, p)
        uc = np.ascontiguousarray(uc.transpose(0, 4, 1, 3, 2)).reshape(
            nchunk, 128, W * 64
        )
        # h0: (p, (m, b))
        hc = hidden[c * BC : (c + 1) * BC, :]  # [32, 256] dims (b, (m, p))
        hc = hc.reshape(BC, 2, 128).transpose(2, 1, 0)  # (p, m, b)
        hc = np.ascontiguousarray(hc).reshape(128, 64)
        in_maps.append({"u": uc, "h0": hc, "wt": wt_s, "ident": ident})

    trace = bool(int(os.environ.get("KERNEL_TRACE", "0")))
    if trace:
        _install_profile_hook()
    nc = _get_nc(T, W)
    res = run_bass_kernel_spmd(
        nc,
        in_maps,
        core_ids=list(range(N_CORES)),
        trace=trace,
    )
    kernel.last_results = res

    # ---- host: unpack hidden_list, compute output head ----
    hidden_list = np.empty((B, T, NHID), dtype=np.float32)
    for c in range(N_CORES):
        arr = res.results[c]["hout"].reshape(nchunk, 128, W, 2, BC)
        # (chunk, p, w, m, b) -> (b, chunk, w, m, p)
        arr = arr.transpose(4, 0, 2, 3, 1).reshape(BC, T, NHID)
        hidden_list[c * BC : (c + 1) * BC] = arr

    hidden_final = np.ascontiguousarray(hidden_list[:, -1, :])
    out = hidden_list.reshape(B * T, NHID) @ w_out_w.T.astype(np.float32)
    output_list = (20.0 * np.tanh(out)).reshape(B, T, NOUT).astype(np.float32)
    return hidden_list, output_list, hidden_final


# revision 14
# speedup vs baseline: 1.0141x; 1.0003x over previous
"""Trainium2 Bass kernel for the noisy-RNN recurrence problem.

Reference computation (per step t):
    tmp   = x_t @ w_in^T + tanh(h) @ w_hh^T + b
    h_new = (1-a)*h + a*tmp + noise_t * (sigma*sqrt(a))
    out   = 20*tanh(h_new @ w_out^T)
Outputs: (hidden_list [B,T,NHID], output_list [B,T,NOUT], hidden_final [B,NHID])

Strategy:
  * Data-parallel over batch: 8 cores x 32 batch each.
  * Host pre-folds u_t = a*(x_t @ w_in^T) + a*b + sigma*sqrt(a)*noise_t into one
    fp32 stream (the dominant memory traffic), pre-transposed into the on-chip
    layout so no on-device transposes are needed.
  * On device, state h^T is kept as a [128 part, (chain, m, b)] fp32 tile
    group; per step: tanh (ScalarE, fp16 out) -> 4 matmuls (fp16 weights
    stationary, fp32 PSUM) -> psum+u (DVE) -> h_new = 0.75*h + t1 (DVE fused
    scalar_tensor_tensor).  Two independent 16-batch chains per core pipeline
    the serial recurrence across engines.
  * output_list and hidden_final are pure functions of hidden_list; computed
    on host (fp32) to keep device traffic at the memory roofline.
"""

import os
import sys

import numpy as np

for _p in ("/opt/trn_rl_repo",):
    if _p not in sys.path:
        sys.path.insert(0, _p)

import concourse.bass as bass  # noqa: E402
import concourse.tile as tile  # noqa: E402
from concourse import bacc, mybir  # noqa: E402
from concourse.bass_utils import run_bass_kernel_spmd  # noqa: E402

ALPHA = np.float32(0.25)
SIGMA_NEU = np.float32(0.05)
N_CORES = 8
W = 16  # timesteps per DMA chunk

_BUILD_CACHE = {}


def _install_profile_hook():
    """Provide antenv.axon_hooks (NTFF profiling) if the image lacks it."""
    try:
        import antenv.axon_hooks  # noqa: F401

        return
    except ImportError:
        pass
    import contextlib
    import ctypes
    import types

    so = "/opt/axon/libaxon_pjrt.so"
    if not os.path.exists(so):
        return
    lib = ctypes.CDLL(so)
    if not hasattr(lib, "axon_start_nrt_profile"):
        return
    lib.axon_start_nrt_profile.argtypes = [
        ctypes.POINTER(ctypes.c_int64),
        ctypes.c_size_t,
    ]
    lib.axon_start_nrt_profile.restype = ctypes.c_int64
    lib.axon_stop_nrt_profile.argtypes = [ctypes.c_char_p]
    lib.axon_stop_nrt_profile.restype = ctypes.c_int64

    @contextlib.contextmanager
    def _hook(output_dir, device_ids):
        import jax

        jax.devices()
        if device_ids:
            ids = (ctypes.c_int64 * len(device_ids))(*device_ids)
            rc = lib.axon_start_nrt_profile(ids, len(device_ids))
        else:
            rc = lib.axon_start_nrt_profile(None, 0)
        if rc != 0:
            raise RuntimeError(f"axon_start_nrt_profile rc={rc}")
        try:
            yield
        finally:
            n = lib.axon_stop_nrt_profile(str(output_dir).encode())
            print(f"profile: {n} file(s) written to {output_dir}", file=sys.stderr)

    import antenv

    mod = types.ModuleType("antenv.axon_hooks")
    mod.get_axon_ntff_profile_hook = lambda: _hook
    mod.set_axon_ntff_profile_hook = lambda h: None
    antenv.axon_hooks = mod
    sys.modules["antenv.axon_hooks"] = mod


S_FP16 = True  # dtype of the s = 0.75*h + u path fed back via identity-matmul


def _build(T, W):
    """Build + compile the per-core bass program (SPMD across 8 cores).

    Design: the state h lives in PSUM. One chain over the full 32-batch,
    tile free layout (m, b) = m*32+b (nhid half m, batch b).
    Per step t (psum slice `sl` holds h_t):
      - ScalarE: th_t = tanh(psum[sl]) -> fp16        (parallel with DVE)
      - DVE:     s_t  = 0.75*psum[sl] + u_t -> fp16/f32
      - TensorE: psum[sl+1] = sum_k aW^T_k @ th_t  +  I @ s_t
    Every 8 steps one PSUM bank completes -> DVE copies it to SBUF hist,
    DMA'd out every 16 steps. hidden_list <- hist; output head on host.
    """
    nchunk = T // W
    assert W == 16
    f32, f16 = mybir.dt.float32, mybir.dt.float16
    sdt = f16 if S_FP16 else f32
    nc = bacc.Bacc("TRN2", target_bir_lowering=False, debug=False)
    u_d = nc.dram_tensor("u", [nchunk, 128, W * 64], f32, kind="ExternalInput").ap()
    h0_d = nc.dram_tensor("h0", [128, 64], f32, kind="ExternalInput").ap()
    wt_d = nc.dram_tensor("wt", [128, 512], f16, kind="ExternalInput").ap()
    id_d = nc.dram_tensor("ident", [128, 128], sdt, kind="ExternalInput").ap()
    ho_d = nc.dram_tensor("hout", [nchunk, 128, W * 64], f32, kind="ExternalOutput").ap()

    Tanh = mybir.ActivationFunctionType.Tanh
    add, mult = mybir.AluOpType.add, mybir.AluOpType.mult

    with tile.TileContext(nc) as tc:
        with (
            tc.tile_pool(name="const", bufs=1) as constp,
            tc.tile_pool(name="upool", bufs=3) as upool,
            tc.tile_pool(name="hist", bufs=3) as histp,
            tc.tile_pool(name="th", bufs=3) as thp,
            tc.tile_pool(name="sp", bufs=3) as sp,
            tc.tile_pool(name="zp", bufs=3) as zp,
            tc.tile_pool(name="ps", bufs=3, space="PSUM") as psp,
            tc.tile_pool(name="pa", bufs=3, space="PSUM") as pap,
        ):
            wt_s = constp.tile([128, 512], f16)
            nc.sync.dma_start(wt_s[:], wt_d[:, :])
            id_s = constp.tile([128, 128], sdt)
            nc.sync.dma_start(id_s[:], id_d[:, :])
            h0_s = constp.tile([128, 64], f32)
            nc.sync.dma_start(h0_s[:], h0_d[:, :])

            # One PSUM bank tile [128,64] per step: step t writes h_{t+1}
            # into its own bank (w-matmuls first - gated only on th_t - then
            # the identity-matmul injecting s_t last).  Readers of bank t
            # (tanh, stt, hist-copy) never share a bank with the writers of
            # bank t+1, so PE writes overlap the reads of the previous step.
            # Two PSUM banks per step: bank_H[i] = h_{i+1} (I-mm of s_i first,
            # then 4 w-matmuls of th_i); bank_A[i] = A_{i+1} (w-matmuls only,
            # lower priority).  ScalarE's tanh reads bank_H, VectorE's s-path
            # reads bank_A - different banks, so they run in parallel instead
            # of being serialized by Tile's same-bank ordering.
            #   z_i = 0.75*s_{i-1} + u_i          (DVE, SBUF only, off-chain)
            #   s_i = 0.75*A_i + z_i              (DVE, reads bank_A[i-1])
            read_ap = h0_s[:]
            a_prev = None
            s_prev = None
            u_s = None
            hist = None
            pend = None

            def flush_pend():
                src, ht, col, dma_chunk = pend
                nc.vector.tensor_copy(ht[:, col : col + 64], src)
                if dma_chunk is not None:
                    nc.sync.dma_start(ho_d[dma_chunk], ht[:])

            for t in range(T):
                w16 = t % 16
                if w16 == 0:
                    u_s = upool.tile([128, W * 64], f32)
                    nc.sync.dma_start(u_s[:], u_d[t // W])
                    hist = histp.tile([128, W * 64], f32)
                th = thp.tile([128, 64], f16)
                nc.scalar.activation(th[:], read_ap, Tanh)
                u_ap = u_s[:, w16 * 64 : (w16 + 1) * 64]
                s = sp.tile([128, 64], sdt)
                if t == 0:
                    nc.vector.scalar_tensor_tensor(
                        s[:], h0_s[:], 0.75, u_ap, mult, add
                    )
                else:
                    z = zp.tile([128, 64], f32)
                    nc.vector.scalar_tensor_tensor(
                        z[:], s_prev[:], 0.75, u_ap, mult, add
                    )
                    nc.vector.scalar_tensor_tensor(
                        s[:], a_prev[:], 0.75, z[:], mult, add
                    )
                if pend is not None:
                    flush_pend()
                ps = psp.tile([128, 64], f32)
                nc.tensor.matmul(
                    ps[:], id_s[:], s[:],
                    start=True, stop=False, skip_group_check=True,
                )
                for m in range(2):
                    for k in range(2):
                        nc.tensor.matmul(
                            ps[:, m * 32 : (m + 1) * 32],
                            wt_s[:, k * 256 + m * 128 : k * 256 + (m + 1) * 128],
                            th[:, k * 32 : (k + 1) * 32],
                            start=False,
                            stop=(m == 1 and k == 1),
                            skip_group_check=True,
                        )
                if t < T - 1:
                    pa = pap.tile([128, 64], f32)
                    for m in range(2):
                        for k in range(2):
                            nc.tensor.matmul(
                                pa[:, m * 32 : (m + 1) * 32],
                                wt_s[:, k * 256 + m * 128 : k * 256 + (m + 1) * 128],
                                th[:, k * 32 : (k + 1) * 32],
                                start=(k == 0),
                                stop=(k == 1),
                                skip_group_check=True,
                            )
                    a_prev = pa
                s_prev = s
                read_ap = ps[:]
                pend = (ps[:], hist, w16 * 64, t // W if w16 == 15 else None)
            flush_pend()
    nc.compile()
    return nc


def _get_nc(T, W):
    key = (T, W)
    if key not in _BUILD_CACHE:
        _BUILD_CACHE[key] = _build(T, W)
    return _BUILD_CACHE[key]


def kernel(input_signal, hidden, w_in_w, w_hh_w, w_hh_b, w_out_w, noise, length):
    input_signal = np.asarray(input_signal, dtype=np.float32)
    hidden = np.asarray(hidden, dtype=np.float32)
    w_in_w = np.asarray(w_in_w, dtype=np.float32)
    w_hh_w = np.asarray(w_hh_w, dtype=np.float32)
    w_hh_b = np.asarray(w_hh_b, dtype=np.float32)
    w_out_w = np.asarray(w_out_w, dtype=np.float32)
    noise = np.asarray(noise, dtype=np.float32)

    B, T, NIN = input_signal.shape
    NHID = hidden.shape[1]
    NOUT = w_out_w.shape[0]
    BC = B // N_CORES  # 32 batch per core
    nchunk = T // W
    sigp = SIGMA_NEU * np.sqrt(ALPHA)

    # ---- host: fold input projection + bias + noise into one fp32 stream ----
    # u_full[t, b, :] = a*(x[b,t,:] @ w_in^T) + a*b_hh + sigp*noise[t,b,:]
    xt = np.ascontiguousarray(input_signal.transpose(1, 0, 2)).reshape(T * B, NIN)
    proj = (xt @ (ALPHA * w_in_w.T)).reshape(T, B, NHID)
    u_full = proj + (ALPHA * w_hh_b)[None, None, :] + sigp * noise
    u_full = np.ascontiguousarray(u_full, dtype=np.float32)

    # weights, stationary layout: wt_s[p, k*256 + j] = (a*w_hh^T)[k*128+p, j]
    wt = (ALPHA * w_hh_w.T).astype(np.float16)  # [256 (i=contraction), 256 (j=out)]
    wt_s = np.ascontiguousarray(
        wt.reshape(2, 128, 256).transpose(1, 0, 2).reshape(128, 512)
    )

    ident = np.eye(128, dtype=np.float16 if S_FP16 else np.float32)
    in_maps = []
    for c in range(N_CORES):
        # u: (chunk, p, (w, m, b))
        uc = u_full[:, c * BC : (c + 1) * BC, :]  # [T, 32, 256]
        uc = uc.reshape(nchunk, W, BC, 2, 128)  # (chunk, w, b, m, p)
        uc = np.ascontiguousarray(uc.transpose(0, 4, 1, 3, 2)).reshape(
            nchunk, 128, W * 64
        )
        # h0: (p, (m, b))
        hc = hidden[c * BC : (c + 1) * BC, :]  # [32, 256] dims (b, (m, p))
        hc = hc.reshape(BC, 2, 128).transpose(2, 1, 0)  # (p, m, b)
        hc = np.ascontiguousarray(hc).reshape(128, 64)
        in_maps.append({"u": uc, "h0": hc, "wt": wt_s, "ident": ident})

    trace = bool(int(os.environ.get("KERNEL_TRACE", "0")))
    if trace:
        _install_profile_hook()
    nc = _get_nc(T, W)
    res = run_bass_kernel_spmd(
        nc,
        in_maps,
        core_ids=list(range(N_CORES)),
        trace=trace,
    )
    kernel.last_results = res

    # ---- host: unpack hidden_list, compute output head ----
    hidden_list = np.empty((B, T, NHID), dtype=np.float32)
    for c in range(N_CORES):
        arr = res.results[c]["hout"].reshape(nchunk, 128, W, 2, BC)
        # (chunk, p, w, m, b) -> (b, chunk, w, m, p)
        arr = arr.transpose(4, 0, 2, 3, 1).reshape(BC, T, NHID)
        hidden_list[c * BC : (c + 1) * BC] = arr

    hidden_final = np.ascontiguousarray(hidden_list[:, -1, :])
    out = hidden_list.reshape(B * T, NHID) @ w_out_w.T.astype(np.float32)
    output_list = (20.0 * np.tanh(out)).reshape(B, T, NOUT).astype(np.float32)
    return hidden_list, output_list, hidden_final
